# revision 18
# baseline (speedup 1.0000x reference)
"""2-layer GCN on 8 trn2 NeuronCores — single fused SPMD launch.

Full inputs in, full outputs out. Host sorts edges by dst and packs them
into groups of <=128 dst-nodes / <=2048 edges (16 tiles of 128). Each core
owns a contiguous run of groups (balanced by edge count) plus 1/8 of the
nodes for the dense layer. Per-tile segment-sum is a TensorE matmul with an
on-device-built one-hot*(norm) selection matrix, accumulated in PSUM.

One launch does everything on device:
  A: S0_c = X_c @ W0.T          (node-sharded)      -> AllGather S0
  B: H_c  = relu(seg_sum(S0[src]*norm, dst))        -> AllGather H
  C: Z_c  = seg_sum(H[src]*norm, dst) @ W1.T        (stored transposed)

src indices are pre-remapped on the host into positions in the
all-gathered (padded, core-major) S0/H layouts; the two remaps are packed
into one int32 (layer1 | layer2<<16) and unpacked on device. Edge slot ids
and norms ship as bf16, intermediates and outputs are bf16 — host<->device
tunnel traffic dominates wall time.

The PJRT executable is AOT-compiled at build time (persistent jax
compilation cache + neuron NEFF cache make this fast on repeat runs); the
timed section is transfer + execute + readback only.
"""

import os
import time

import numpy as np
from ml_dtypes import bfloat16

import jax

jax.config.update("jax_compilation_cache_dir",
                  os.path.expanduser("~/.jax_comp_cache"))
jax.config.update("jax_persistent_cache_min_entry_size_bytes", -1)
jax.config.update("jax_persistent_cache_min_compile_time_secs", 0)

import jax.numpy as jnp
from jax.sharding import Mesh, NamedSharding, PartitionSpec
from jax.experimental.shard_map import shard_map

import concourse.bacc as bacc
import concourse.bass as bass
import concourse.bass2jax as b2j
import concourse.tile as tile
from concourse import mybir

P = 128
TPG = 16                 # tiles (of 128 edges) per group
EPG = P * TPG            # 2048 edge slots per group
NCORES = 8
N = 50000
D = 128
RPC = N // NCORES        # 6250 node rows per core (exact)
CHA = -(-RPC // P)       # 49 row-tiles per core in phase A
RPAD = CHA * P           # 6272 padded rows per core
F32 = mybir.dt.float32
BF16 = mybir.dt.bfloat16
I32 = mybir.dt.int32
I8 = mybir.dt.int8

LAST_TIMES = {}


def _pack_groups(dst_sorted):
    """Greedy pack sorted dst nodes into groups (<=P nodes, <=EPG edges).
    Returns list of (edge_start, edge_cnt, node_ids ndarray)."""
    nodes, counts = np.unique(dst_sorted, return_counts=True)
    groups = []
    i, e = 0, 0
    nn = len(nodes)
    while i < nn:
        es = e
        ns = i
        cnt_e = 0
        while i < nn and (i - ns) < P and cnt_e + counts[i] <= EPG:
            cnt_e += int(counts[i])
            i += 1
        assert i > ns, "single node exceeds group capacity"
        e += cnt_e
        groups.append((es, cnt_e, nodes[ns:i]))
    return groups


def _build_fused(G):
    """G = max real groups per core. h_loc gets one extra all-zero group so
    its first row doubles as the gather target for srcs with no in-edges."""
    G1 = G + 1
    nc = bacc.Bacc(None, target_bir_lowering=False, num_swdge_queues=4,
                   num_devices=NCORES)
    # X ships int8, quantized per feature dim; the dequant scales are folded
    # into w0t rows on the host so the device only does an i8->bf16 copy
    xt = nc.declare_dram_parameter("xt", [D, RPAD], I8, isOutput=False)
    w0t = nc.declare_dram_parameter("w0t", [D, D], BF16, isOutput=False)
    w1t = nc.declare_dram_parameter("w1t", [D, D], BF16, isOutput=False)
    pidx = nc.declare_dram_parameter("pidx", [G, P, TPG], I32, isOutput=False)
    slot = nc.declare_dram_parameter("slot", [G, P, TPG], I8, isOutput=False)
    sn = nc.declare_dram_parameter("sn", [G, P, TPG], BF16, isOutput=False)
    # single fused output (H rows then Z.T rows), int8 with per-row f32
    # scales: halves the dominant d2h stream vs bf16 at ~0.7% added error
    hz = nc.declare_dram_parameter("hz", [2 * G * P, D], I8, isOutput=True)
    hsc = nc.declare_dram_parameter("hsc", [2 * G * P, 1], F32, isOutput=True)

    with tile.TileContext(nc) as tc:
        with (
            tc.tile_pool(name="dram", bufs=1, space="DRAM") as dram,
            tc.tile_pool(name="const", bufs=1) as cpool,
            tc.tile_pool(name="sbuf", bufs=4) as pool,
            tc.tile_pool(name="psum", bufs=2, space="PSUM") as psum,
            tc.tile_pool(name="psum2", bufs=2, space="PSUM") as psum2,
        ):
            s0_loc = dram.tile([RPAD, D], BF16)
            s0_full = dram.tile([NCORES * RPAD, D], BF16)
            h_loc = dram.tile([G1 * P, D], BF16)
            h_full = dram.tile([NCORES * G1 * P, D], BF16)

            iota_i = cpool.tile([P, P], dtype=I32)
            nc.gpsimd.iota(iota_i[:], pattern=[[1, P]], base=0,
                           channel_multiplier=0)
            iota_sb = cpool.tile([P, P], dtype=BF16)
            nc.vector.tensor_copy(iota_sb[:], iota_i[:])
            zrow_sb = cpool.tile([P, D], dtype=BF16)
            nc.vector.memset(zrow_sb[:], 0.0)
            w0t_sb = cpool.tile([D, D], dtype=BF16)
            nc.sync.dma_start(out=w0t_sb[:], in_=w0t[:])
            w1t_sb = cpool.tile([D, D], dtype=BF16)
            nc.sync.dma_start(out=w1t_sb[:], in_=w1t[:])

            # ---- phase A: S0_c = X_c @ W0.T (X arrives transposed) ----
            for t in range(CHA):
                xq_sb = pool.tile([P, P], dtype=I8, tag="xq")
                nc.sync.dma_start(out=xq_sb[:], in_=xt[:, t * P:(t + 1) * P])
                xt_sb = pool.tile([P, P], dtype=BF16, tag="xt")
                nc.vector.tensor_copy(xt_sb[:], xq_sb[:])
                s_ps = psum.tile([P, D], dtype=F32, tag="s")
                nc.tensor.matmul(out=s_ps[:], lhsT=xt_sb[:], rhs=w0t_sb[:],
                                 start=True, stop=True)
                s_sb = pool.tile([P, D], dtype=BF16, tag="s0")
                nc.vector.tensor_copy(s_sb[:], s_ps[:])
                nc.sync.dma_start(out=s0_loc[t * P:(t + 1) * P, :], in_=s_sb[:])

            nc.gpsimd.collective_compute(
                "AllGather", mybir.AluOpType.bypass,
                replica_groups=[list(range(NCORES))],
                ins=[s0_loc[:].opt()], outs=[s0_full[:].opt()],
            )

            # ---- phase B: H = relu(seg_sum(S0[src]*norm, dst)) ----
            nc.sync.dma_start(out=h_loc[G * P:G1 * P, :], in_=zrow_sb[:])
            for g in range(G):
                pidx_sb = pool.tile([P, TPG], dtype=I32, tag="pidx")
                nc.sync.dma_start(out=pidx_sb[:], in_=pidx[g])
                idx_sb = pool.tile([P, TPG], dtype=I32, tag="idx")
                nc.vector.tensor_scalar(
                    out=idx_sb[:], in0=pidx_sb[:], scalar1=0xFFFF, scalar2=None,
                    op0=mybir.AluOpType.bitwise_and)
                sl8_sb = pool.tile([P, TPG], dtype=I8, tag="sl8")
                nc.sync.dma_start(out=sl8_sb[:], in_=slot[g])
                sl_sb = pool.tile([P, TPG], dtype=BF16, tag="sl")
                nc.vector.tensor_copy(sl_sb[:], sl8_sb[:])
                sn_sb = pool.tile([P, TPG], dtype=BF16, tag="sn")
                nc.sync.dma_start(out=sn_sb[:], in_=sn[g])
                nrm_sb = pool.tile([P, TPG], dtype=F32, tag="nrm")
                nc.vector.tensor_copy(nrm_sb[:], sn_sb[:])
                acc_ps = psum.tile([P, D], dtype=F32, tag="acc")
                for t in range(TPG):
                    g_sb = pool.tile([P, D], dtype=BF16, tag="gat")
                    nc.gpsimd.indirect_dma_start(
                        out=g_sb[:], out_offset=None, in_=s0_full[:],
                        in_offset=bass.IndirectOffsetOnAxis(
                            ap=idx_sb[:, t:t + 1], axis=0),
                    )
                    sel = pool.tile([P, P], dtype=BF16, tag="sel")
                    nc.vector.tensor_tensor(
                        out=sel[:], in0=sl_sb[:, t:t + 1].to_broadcast([P, P])[:],
                        in1=iota_sb[:], op=mybir.AluOpType.is_equal,
                    )
                    pm = pool.tile([P, P], dtype=BF16, tag="pm")
                    nc.vector.tensor_scalar_mul(
                        pm[:], sel[:], nrm_sb[:, t:t + 1])
                    nc.tensor.matmul(out=acc_ps[:], lhsT=pm[:], rhs=g_sb[:],
                                     start=(t == 0), stop=(t == TPG - 1))
                h_sb = pool.tile([P, D], dtype=BF16, tag="h")
                nc.scalar.activation(h_sb[:], acc_ps[:],
                                     mybir.ActivationFunctionType.Relu)
                nc.sync.dma_start(out=h_loc[g * P:(g + 1) * P, :], in_=h_sb[:])
                # int8-quantize H rows (relu output >= 0, so max == absmax)
                m_sb = pool.tile([P, 1], dtype=F32, tag="m")
                nc.vector.reduce_max(m_sb[:], h_sb[:], axis=mybir.AxisListType.X)
                s_sb = pool.tile([P, 1], dtype=F32, tag="s")
                nc.scalar.activation(s_sb[:], m_sb[:],
                                     mybir.ActivationFunctionType.Copy,
                                     bias=1e-20, scale=1.0 / 127.0)
                qs_sb = pool.tile([P, 1], dtype=F32, tag="qs")
                nc.vector.reciprocal(qs_sb[:], s_sb[:])
                q_sb = pool.tile([P, D], dtype=I8, tag="q")
                nc.vector.tensor_scalar_mul(q_sb[:], h_sb[:], qs_sb[:, 0:1])
                nc.sync.dma_start(out=hz[g * P:(g + 1) * P, :], in_=q_sb[:])
                nc.sync.dma_start(out=hsc[g * P:(g + 1) * P, :], in_=s_sb[:])

            nc.gpsimd.collective_compute(
                "AllGather", mybir.AluOpType.bypass,
                replica_groups=[list(range(NCORES))],
                ins=[h_loc[:].opt()], outs=[h_full[:].opt()],
            )

            # ---- phase C: Z = seg_sum(H[src]*norm, dst) @ W1.T ----
            # Accumulate transposed (accT = gathered.T @ pm) so the final
            # matmul zT = w1t.T @ accT needs no PE transpose. zout holds
            # Z_g.T per group; the host transposes back.
            for g in range(G):
                pidx_sb = pool.tile([P, TPG], dtype=I32, tag="pidx")
                nc.sync.dma_start(out=pidx_sb[:], in_=pidx[g])
                idx_sb = pool.tile([P, TPG], dtype=I32, tag="idx")
                nc.vector.tensor_scalar(
                    out=idx_sb[:], in0=pidx_sb[:], scalar1=16, scalar2=None,
                    op0=mybir.AluOpType.logical_shift_right)
                sl8_sb = pool.tile([P, TPG], dtype=I8, tag="sl8")
                nc.sync.dma_start(out=sl8_sb[:], in_=slot[g])
                sl_sb = pool.tile([P, TPG], dtype=BF16, tag="sl")
                nc.vector.tensor_copy(sl_sb[:], sl8_sb[:])
                sn_sb = pool.tile([P, TPG], dtype=BF16, tag="sn")
                nc.sync.dma_start(out=sn_sb[:], in_=sn[g])
                nrm_sb = pool.tile([P, TPG], dtype=F32, tag="nrm")
                nc.vector.tensor_copy(nrm_sb[:], sn_sb[:])
                acc_ps = psum.tile([P, P], dtype=F32, tag="acc")
                for t in range(TPG):
                    g_sb = pool.tile([P, D], dtype=BF16, tag="gat")
                    nc.gpsimd.indirect_dma_start(
                        out=g_sb[:], out_offset=None, in_=h_full[:],
                        in_offset=bass.IndirectOffsetOnAxis(
                            ap=idx_sb[:, t:t + 1], axis=0),
                    )
                    sel = pool.tile([P, P], dtype=BF16, tag="sel")
                    nc.vector.tensor_tensor(
                        out=sel[:], in0=sl_sb[:, t:t + 1].to_broadcast([P, P])[:],
                        in1=iota_sb[:], op=mybir.AluOpType.is_equal,
                    )
                    pm = pool.tile([P, P], dtype=BF16, tag="pm")
                    nc.vector.tensor_scalar_mul(
                        pm[:], sel[:], nrm_sb[:, t:t + 1])
                    nc.tensor.matmul(out=acc_ps[:], lhsT=g_sb[:], rhs=pm[:],
                                     start=(t == 0), stop=(t == TPG - 1))
                at_sb = pool.tile([P, P], dtype=BF16, tag="aT")
                nc.vector.tensor_copy(at_sb[:], acc_ps[:])
                z_ps = psum2.tile([P, P], dtype=F32, tag="zT")
                nc.tensor.matmul(out=z_ps[:], lhsT=w1t_sb[:], rhs=at_sb[:],
                                 start=True, stop=True)
                # int8-quantize Z.T rows (per out-dim within the group)
                m_sb = pool.tile([P, 1], dtype=F32, tag="m")
                nc.vector.reduce_max(m_sb[:], z_ps[:], axis=mybir.AxisListType.X,
                                     apply_absolute_value=True)
                s_sb = pool.tile([P, 1], dtype=F32, tag="s")
                nc.scalar.activation(s_sb[:], m_sb[:],
                                     mybir.ActivationFunctionType.Copy,
                                     bias=1e-20, scale=1.0 / 127.0)
                qs_sb = pool.tile([P, 1], dtype=F32, tag="qs")
                nc.vector.reciprocal(qs_sb[:], s_sb[:])
                q_sb = pool.tile([P, P], dtype=I8, tag="q")
                nc.vector.tensor_scalar_mul(q_sb[:], z_ps[:], qs_sb[:, 0:1])
                nc.sync.dma_start(out=hz[(G + g) * P:(G + g + 1) * P, :],
                                  in_=q_sb[:])
                nc.sync.dma_start(out=hsc[(G + g) * P:(G + g + 1) * P, :],
                                  in_=s_sb[:])
    nc.compile()
    return nc


def _prepare_exec(nc):
    """AOT-compile the SPMD executable (mirrors run_bass_via_pjrt, but with
    lowering/compilation split out so the timed section is exec-only), and
    materialize the donated zero output buffers directly on device."""
    b2j.install_neuronx_cc_hook()
    partition_name = nc.partition_id_tensor.name if nc.partition_id_tensor else None
    in_names, out_names, out_avals, zero_shapes = [], [], [], []
    for alloc in nc.m.functions[0].allocations:
        if not isinstance(alloc, mybir.MemoryLocationSet):
            continue
        name = alloc.memorylocations[0].name
        if alloc.kind == "ExternalInput":
            if name != partition_name:
                in_names.append(name)
        elif alloc.kind == "ExternalOutput":
            out_names.append(name)
            shape = tuple(alloc.tensor_shape)
            dtype = mybir.dt.np(alloc.dtype)
            out_avals.append(jax.core.ShapedArray(shape, dtype))
            zero_shapes.append((shape, dtype))
    n_params = len(in_names)
    n_outs = len(out_avals)
    in_names = in_names + out_names
    if partition_name is not None:
        in_names.append(partition_name)
    donate = tuple(range(n_params, n_params + n_outs))

    def _body(*args):
        operands = list(args)
        if partition_name is not None:
            operands.append(b2j.partition_id_tensor())
        outs = b2j._bass_exec_p.bind(
            *operands, out_avals=tuple(out_avals), in_names=tuple(in_names),
            out_names=tuple(out_names), lowering_input_output_aliases=(),
            sim_require_finite=True, sim_require_nnan=True, nc=nc)
        return tuple(outs)

    devices = jax.devices()[:NCORES]
    mesh = Mesh(np.asarray(devices), ("core",))
    spec = PartitionSpec("core")
    in_specs = (spec,) * (n_params + n_outs)
    out_specs = (spec,) * n_outs
    sharded = jax.jit(
        shard_map(_body, mesh=mesh, in_specs=in_specs, out_specs=out_specs,
                  check_rep=False),
        donate_argnums=donate, keep_unused=True)

    def g_struct(shape, dtype):
        return jax.ShapeDtypeStruct((NCORES * shape[0], *shape[1:]), dtype)

    in_structs = []
    # parameter avals in declaration order, via the module allocations again
    shapes_by_name = {}
    for alloc in nc.m.functions[0].allocations:
        if isinstance(alloc, mybir.MemoryLocationSet) and alloc.kind == "ExternalInput":
            shapes_by_name[alloc.memorylocations[0].name] = (
                tuple(alloc.tensor_shape), mybir.dt.np(alloc.dtype))
    for name in in_names[:n_params]:
        shp, dt = shapes_by_name[name]
        in_structs.append(g_struct(shp, dt))
    zero_structs = [g_struct(shp, dt) for shp, dt in zero_shapes]
    compiled = sharded.lower(*in_structs, *zero_structs).compile()

    sharding = NamedSharding(mesh, spec)
    zeros_dev = [
        jax.jit(lambda s=shp, d=dt: jnp.zeros((NCORES * s[0], *s[1:]), d),
                out_shardings=sharding)()
        for shp, dt in zero_shapes]
    jax.block_until_ready(zeros_dev)
    return compiled, in_names[:n_params], out_names, out_avals, zeros_dev


def kernel(X, W0, W1, norm, src, dst):
    t0 = time.perf_counter()
    X = np.asarray(X, dtype=np.float32)
    W0 = np.asarray(W0, dtype=np.float32)
    W1 = np.asarray(W1, dtype=np.float32)
    norm = np.asarray(norm, dtype=np.float32)
    src = np.asarray(src).astype(np.int64)
    dst = np.asarray(dst).astype(np.int64)
    E = src.shape[0]

    # ---- host preprocessing: sort by dst, pack groups, shard to cores ----
    order = np.argsort(dst, kind="stable")
    src_s = src[order].astype(np.int32)
    dst_s = dst[order]
    norm_s = norm[order]
    groups = _pack_groups(dst_s)
    cum = np.cumsum([g[1] for g in groups])
    core_of = np.minimum((NCORES * (cum - 1) // E).astype(np.int64), NCORES - 1)
    per_core = [[] for _ in range(NCORES)]
    for gi, g in enumerate(groups):
        per_core[int(core_of[gi])].append(g)
    G = max(len(lst) for lst in per_core)
    G1 = G + 1

    # src remaps into the all-gathered padded layouts
    pos1 = ((src_s // RPC) * RPAD + (src_s % RPC)).astype(np.int32)
    pos2_map = np.full(N, G * P, dtype=np.int32)  # default: zero row

    pidx_arr = np.zeros((NCORES, G, P, TPG), dtype=np.int32)
    slot_arr = np.full((NCORES, G, P, TPG), -1, dtype=np.int8)
    sn_arr = np.zeros((NCORES, G, P, TPG), dtype=bfloat16)
    asm_rows, asm_ids = [], []
    for c in range(NCORES):
        rows_l, ids_l = [], []
        for g_i, (es, ce, node_ids) in enumerate(per_core[c]):
            d_loc = np.searchsorted(node_ids, dst_s[es:es + ce]).astype(np.float32)
            j = np.arange(ce)
            t_i, p_i = j // P, j % P
            pidx_arr[c, g_i, p_i, t_i] = pos1[es:es + ce]
            slot_arr[c, g_i, p_i, t_i] = d_loc.astype(np.int8)
            sn_arr[c, g_i, p_i, t_i] = norm_s[es:es + ce].astype(bfloat16)
            pos2_map[node_ids] = c * G1 * P + g_i * P + np.arange(len(node_ids))
            rows_l.append(g_i * P + np.arange(len(node_ids)))
            ids_l.append(node_ids)
        asm_rows.append(np.concatenate(rows_l) if rows_l else np.zeros(0, np.int64))
        asm_ids.append(np.concatenate(ids_l) if ids_l else np.zeros(0, np.int64))
    # layer-2 gathers use the same edge slots; pack both remaps in one int32
    pos2 = pos2_map[src_s]
    for c in range(NCORES):
        for g_i, (es, ce, node_ids) in enumerate(per_core[c]):
            j = np.arange(ce)
            pidx_arr[c, g_i, j % P, j // P] |= pos2[es:es + ce] << 16

    # quantize X per feature dim; fold the dequant scales into W0T rows so
    # the device never sees them
    xsc = (np.abs(X).max(axis=0) / 127.0 + 1e-20).astype(np.float32)
    Xq = np.round(X / xsc).astype(np.int8)
    W0T = np.ascontiguousarray(W0.T * xsc[:, None]).astype(bfloat16)
    W1T = np.ascontiguousarray(W1.T).astype(bfloat16)
    # per-core X shard, padded and transposed: [D, RPAD] int8
    Xpad = np.zeros((NCORES, RPAD, D), dtype=np.int8)
    Xpad[:, :RPC] = Xq.reshape(NCORES, RPC, D)
    XT = np.ascontiguousarray(Xpad.transpose(0, 2, 1))
    LAST_TIMES["prep_s"] = time.perf_counter() - t0

    t1 = time.perf_counter()
    nc = _build_fused(G)
    compiled, in_names, out_names, out_avals, zeros_dev = _prepare_exec(nc)
    LAST_TIMES["build_s"] = time.perf_counter() - t1

    per_core_in = {
        "xt": XT,
        "w0t": np.broadcast_to(W0T, (NCORES, D, D)),
        "w1t": np.broadcast_to(W1T, (NCORES, D, D)),
        "pidx": pidx_arr,
        "slot": slot_arr,
        "sn": sn_arr,
    }
    concat_in = [np.ascontiguousarray(per_core_in[name]).reshape(
        -1, *per_core_in[name].shape[2:]) for name in in_names]

    t1 = time.perf_counter()
    out_arrs = compiled(*concat_in, *zeros_dev)
    res = [np.asarray(a) for a in out_arrs]
    LAST_TIMES["run_fused_s"] = time.perf_counter() - t1

    hz_q = res[out_names.index("hz")].reshape(NCORES, 2 * G * P, D)
    hsc = res[out_names.index("hsc")].reshape(NCORES, 2 * G * P, 1)
    H = np.zeros((N, D), dtype=np.float32)
    Z = np.zeros((N, D), dtype=np.float32)
    for c in range(NCORES):
        hz_f = hz_q[c].astype(np.float32) * hsc[c]
        H[asm_ids[c]] = hz_f[:G * P][asm_rows[c]]
        zc = hz_f[G * P:].reshape(G, P, P).transpose(0, 2, 1).reshape(G * P, P)
        Z[asm_ids[c]] = zc[asm_rows[c]]

    LAST_TIMES["total_s"] = time.perf_counter() - t0
    return (Z, H)


# revision 19
# speedup vs baseline: 1.4206x; 1.4206x over previous
"""2-layer GCN on 8 trn2 NeuronCores — single fused SPMD launch.

Full inputs in, full outputs out. Host sorts edges by dst and packs them
into groups of <=128 dst-nodes / <=2048 edges (16 tiles of 128). Each core
owns a contiguous run of groups (balanced by edge count) plus 1/8 of the
nodes for the dense layer. Per-tile segment-sum is a TensorE matmul with an
on-device-built one-hot*(norm) selection matrix, accumulated in PSUM.

One launch does everything on device:
  A: S0_c = X_c @ W0.T          (node-sharded)      -> AllGather S0
  B: H_c  = relu(seg_sum(S0[src]*norm, dst))        -> AllGather H
  C: Z_c  = seg_sum(H[src]*norm, dst) @ W1.T        (stored transposed)

src indices are pre-remapped on the host into positions in the
all-gathered (padded, core-major) S0/H layouts; the two remaps are packed
into one int32 (layer1 | layer2<<16) and unpacked on device. Edge slot ids
and norms ship as bf16, intermediates and outputs are bf16 — host<->device
tunnel traffic dominates wall time.

The PJRT executable is AOT-compiled at build time (persistent jax
compilation cache + neuron NEFF cache make this fast on repeat runs); the
timed section is transfer + execute + readback only.
"""

import os
import time

import numpy as np
from ml_dtypes import bfloat16

import jax

jax.config.update("jax_compilation_cache_dir",
                  os.path.expanduser("~/.jax_comp_cache"))
jax.config.update("jax_persistent_cache_min_entry_size_bytes", -1)
jax.config.update("jax_persistent_cache_min_compile_time_secs", 0)

import jax.numpy as jnp
from jax.sharding import Mesh, NamedSharding, PartitionSpec
from jax.experimental.shard_map import shard_map

import concourse.bacc as bacc
import concourse.bass as bass
import concourse.bass2jax as b2j
import concourse.tile as tile
from concourse import mybir

P = 128
TPG = 16                 # tiles (of 128 edges) per group
EPG = P * TPG            # 2048 edge slots per group
NCORES = 8
N = 50000
D = 128
RPC = N // NCORES        # 6250 node rows per core (exact)
CHA = -(-RPC // P)       # 49 row-tiles per core in phase A
RPAD = CHA * P           # 6272 padded rows per core
F32 = mybir.dt.float32
BF16 = mybir.dt.bfloat16
I32 = mybir.dt.int32
I8 = mybir.dt.int8

LAST_TIMES = {}


def _pack_groups(dst_sorted):
    """Greedy pack sorted dst nodes into groups (<=P nodes, <=EPG edges).
    Returns list of (edge_start, edge_cnt, node_ids ndarray)."""
    nodes, counts = np.unique(dst_sorted, return_counts=True)
    groups = []
    i, e = 0, 0
    nn = len(nodes)
    while i < nn:
        es = e
        ns = i
        cnt_e = 0
        while i < nn and (i - ns) < P and cnt_e + counts[i] <= EPG:
            cnt_e += int(counts[i])
            i += 1
        assert i > ns, "single node exceeds group capacity"
        e += cnt_e
        groups.append((es, cnt_e, nodes[ns:i]))
    return groups


def _build_fused(G):
    """G = max real groups per core. h_loc gets one extra all-zero group so
    its first row doubles as the gather target for srcs with no in-edges."""
    G1 = G + 1
    nc = bacc.Bacc(None, target_bir_lowering=False, num_swdge_queues=4,
                   num_devices=NCORES)
    # X ships int8, quantized per feature dim; the dequant scales are folded
    # into w0t rows on the host so the device only does an i8->bf16 copy
    xt = nc.declare_dram_parameter("xt", [D, RPAD], I8, isOutput=False)
    w0t = nc.declare_dram_parameter("w0t", [D, D], BF16, isOutput=False)
    w1t = nc.declare_dram_parameter("w1t", [D, D], BF16, isOutput=False)
    pidx = nc.declare_dram_parameter("pidx", [G, P, TPG], I32, isOutput=False)
    slot = nc.declare_dram_parameter("slot", [G, P, TPG], I8, isOutput=False)
    sn = nc.declare_dram_parameter("sn", [G, P, TPG], BF16, isOutput=False)
    # single fused output (H rows then Z.T rows), int8 with per-row f32
    # scales bit-packed into 4 trailing byte columns: halves the dominant
    # d2h stream vs bf16 at ~0.7% added error, in one contiguous transfer
    hz = nc.declare_dram_parameter("hz", [2 * G * P, D + 4], I8, isOutput=True)

    with tile.TileContext(nc) as tc:
        with (
            tc.tile_pool(name="dram", bufs=1, space="DRAM") as dram,
            tc.tile_pool(name="const", bufs=1) as cpool,
            tc.tile_pool(name="sbuf", bufs=4) as pool,
            tc.tile_pool(name="psum", bufs=2, space="PSUM") as psum,
            tc.tile_pool(name="psum2", bufs=2, space="PSUM") as psum2,
        ):
            s0_loc = dram.tile([RPAD, D], BF16)
            s0_full = dram.tile([NCORES * RPAD, D], BF16)
            h_loc = dram.tile([G1 * P, D], BF16)
            h_full = dram.tile([NCORES * G1 * P, D], BF16)

            iota_i = cpool.tile([P, P], dtype=I32)
            nc.gpsimd.iota(iota_i[:], pattern=[[1, P]], base=0,
                           channel_multiplier=0)
            iota_sb = cpool.tile([P, P], dtype=BF16)
            nc.vector.tensor_copy(iota_sb[:], iota_i[:])
            zrow_sb = cpool.tile([P, D], dtype=BF16)
            nc.vector.memset(zrow_sb[:], 0.0)
            w0t_sb = cpool.tile([D, D], dtype=BF16)
            nc.sync.dma_start(out=w0t_sb[:], in_=w0t[:])
            w1t_sb = cpool.tile([D, D], dtype=BF16)
            nc.sync.dma_start(out=w1t_sb[:], in_=w1t[:])

            # ---- phase A: S0_c = X_c @ W0.T (X arrives transposed) ----
            for t in range(CHA):
                xq_sb = pool.tile([P, P], dtype=I8, tag="xq")
                nc.sync.dma_start(out=xq_sb[:], in_=xt[:, t * P:(t + 1) * P])
                xt_sb = pool.tile([P, P], dtype=BF16, tag="xt")
                nc.vector.tensor_copy(xt_sb[:], xq_sb[:])
                s_ps = psum.tile([P, D], dtype=F32, tag="s")
                nc.tensor.matmul(out=s_ps[:], lhsT=xt_sb[:], rhs=w0t_sb[:],
                                 start=True, stop=True)
                s_sb = pool.tile([P, D], dtype=BF16, tag="s0")
                nc.vector.tensor_copy(s_sb[:], s_ps[:])
                nc.sync.dma_start(out=s0_loc[t * P:(t + 1) * P, :], in_=s_sb[:])

            nc.gpsimd.collective_compute(
                "AllGather", mybir.AluOpType.bypass,
                replica_groups=[list(range(NCORES))],
                ins=[s0_loc[:].opt()], outs=[s0_full[:].opt()],
            )

            # ---- phase B: H = relu(seg_sum(S0[src]*norm, dst)) ----
            nc.sync.dma_start(out=h_loc[G * P:G1 * P, :], in_=zrow_sb[:])
            for g in range(G):
                pidx_sb = pool.tile([P, TPG], dtype=I32, tag="pidx")
                nc.sync.dma_start(out=pidx_sb[:], in_=pidx[g])
                idx_sb = pool.tile([P, TPG], dtype=I32, tag="idx")
                nc.vector.tensor_scalar(
                    out=idx_sb[:], in0=pidx_sb[:], scalar1=0xFFFF, scalar2=None,
                    op0=mybir.AluOpType.bitwise_and)
                sl8_sb = pool.tile([P, TPG], dtype=I8, tag="sl8")
                nc.sync.dma_start(out=sl8_sb[:], in_=slot[g])
                sl_sb = pool.tile([P, TPG], dtype=BF16, tag="sl")
                nc.vector.tensor_copy(sl_sb[:], sl8_sb[:])
                sn_sb = pool.tile([P, TPG], dtype=BF16, tag="sn")
                nc.sync.dma_start(out=sn_sb[:], in_=sn[g])
                nrm_sb = pool.tile([P, TPG], dtype=F32, tag="nrm")
                nc.vector.tensor_copy(nrm_sb[:], sn_sb[:])
                acc_ps = psum.tile([P, D], dtype=F32, tag="acc")
                for t in range(TPG):
                    g_sb = pool.tile([P, D], dtype=BF16, tag="gat")
                    nc.gpsimd.indirect_dma_start(
                        out=g_sb[:], out_offset=None, in_=s0_full[:],
                        in_offset=bass.IndirectOffsetOnAxis(
                            ap=idx_sb[:, t:t + 1], axis=0),
                    )
                    sel = pool.tile([P, P], dtype=BF16, tag="sel")
                    nc.vector.tensor_tensor(
                        out=sel[:], in0=sl_sb[:, t:t + 1].to_broadcast([P, P])[:],
                        in1=iota_sb[:], op=mybir.AluOpType.is_equal,
                    )
                    pm = pool.tile([P, P], dtype=BF16, tag="pm")
                    nc.vector.tensor_scalar_mul(
                        pm[:], sel[:], nrm_sb[:, t:t + 1])
                    nc.tensor.matmul(out=acc_ps[:], lhsT=pm[:], rhs=g_sb[:],
                                     start=(t == 0), stop=(t == TPG - 1))
                h_sb = pool.tile([P, D], dtype=BF16, tag="h")
                nc.scalar.activation(h_sb[:], acc_ps[:],
                                     mybir.ActivationFunctionType.Relu)
                nc.sync.dma_start(out=h_loc[g * P:(g + 1) * P, :], in_=h_sb[:])
                # int8-quantize H rows (relu output >= 0, so max == absmax)
                m_sb = pool.tile([P, 1], dtype=F32, tag="m")
                nc.vector.reduce_max(m_sb[:], h_sb[:], axis=mybir.AxisListType.X)
                s_sb = pool.tile([P, 1], dtype=F32, tag="s")
                nc.scalar.activation(s_sb[:], m_sb[:],
                                     mybir.ActivationFunctionType.Copy,
                                     bias=1e-20, scale=1.0 / 127.0)
                qs_sb = pool.tile([P, 1], dtype=F32, tag="qs")
                nc.vector.reciprocal(qs_sb[:], s_sb[:])
                q_sb = pool.tile([P, D], dtype=I8, tag="q")
                nc.vector.tensor_scalar_mul(q_sb[:], h_sb[:], qs_sb[:, 0:1])
                nc.sync.dma_start(out=hz[g * P:(g + 1) * P, :D], in_=q_sb[:])
                nc.sync.dma_start(out=hz[g * P:(g + 1) * P, D:],
                                  in_=s_sb[:].bitcast(I8))

            nc.gpsimd.collective_compute(
                "AllGather", mybir.AluOpType.bypass,
                replica_groups=[list(range(NCORES))],
                ins=[h_loc[:].opt()], outs=[h_full[:].opt()],
            )

            # ---- phase C: Z = seg_sum(H[src]*norm, dst) @ W1.T ----
            # Accumulate transposed (accT = gathered.T @ pm) so the final
            # matmul zT = w1t.T @ accT needs no PE transpose. zout holds
            # Z_g.T per group; the host transposes back.
            for g in range(G):
                pidx_sb = pool.tile([P, TPG], dtype=I32, tag="pidx")
                nc.sync.dma_start(out=pidx_sb[:], in_=pidx[g])
                idx_sb = pool.tile([P, TPG], dtype=I32, tag="idx")
                nc.vector.tensor_scalar(
                    out=idx_sb[:], in0=pidx_sb[:], scalar1=16, scalar2=None,
                    op0=mybir.AluOpType.logical_shift_right)
                sl8_sb = pool.tile([P, TPG], dtype=I8, tag="sl8")
                nc.sync.dma_start(out=sl8_sb[:], in_=slot[g])
                sl_sb = pool.tile([P, TPG], dtype=BF16, tag="sl")
                nc.vector.tensor_copy(sl_sb[:], sl8_sb[:])
                sn_sb = pool.tile([P, TPG], dtype=BF16, tag="sn")
                nc.sync.dma_start(out=sn_sb[:], in_=sn[g])
                nrm_sb = pool.tile([P, TPG], dtype=F32, tag="nrm")
                nc.vector.tensor_copy(nrm_sb[:], sn_sb[:])
                acc_ps = psum.tile([P, P], dtype=F32, tag="acc")
                for t in range(TPG):
                    g_sb = pool.tile([P, D], dtype=BF16, tag="gat")
                    nc.gpsimd.indirect_dma_start(
                        out=g_sb[:], out_offset=None, in_=h_full[:],
                        in_offset=bass.IndirectOffsetOnAxis(
                            ap=idx_sb[:, t:t + 1], axis=0),
                    )
                    sel = pool.tile([P, P], dtype=BF16, tag="sel")
                    nc.vector.tensor_tensor(
                        out=sel[:], in0=sl_sb[:, t:t + 1].to_broadcast([P, P])[:],
                        in1=iota_sb[:], op=mybir.AluOpType.is_equal,
                    )
                    pm = pool.tile([P, P], dtype=BF16, tag="pm")
                    nc.vector.tensor_scalar_mul(
                        pm[:], sel[:], nrm_sb[:, t:t + 1])
                    nc.tensor.matmul(out=acc_ps[:], lhsT=g_sb[:], rhs=pm[:],
                                     start=(t == 0), stop=(t == TPG - 1))
                at_sb = pool.tile([P, P], dtype=BF16, tag="aT")
                nc.vector.tensor_copy(at_sb[:], acc_ps[:])
                z_ps = psum2.tile([P, P], dtype=F32, tag="zT")
                nc.tensor.matmul(out=z_ps[:], lhsT=w1t_sb[:], rhs=at_sb[:],
                                 start=True, stop=True)
                # int8-quantize Z.T rows (per out-dim within the group)
                m_sb = pool.tile([P, 1], dtype=F32, tag="m")
                nc.vector.reduce_max(m_sb[:], z_ps[:], axis=mybir.AxisListType.X,
                                     apply_absolute_value=True)
                s_sb = pool.tile([P, 1], dtype=F32, tag="s")
                nc.scalar.activation(s_sb[:], m_sb[:],
                                     mybir.ActivationFunctionType.Copy,
                                     bias=1e-20, scale=1.0 / 127.0)
                qs_sb = pool.tile([P, 1], dtype=F32, tag="qs")
                nc.vector.reciprocal(qs_sb[:], s_sb[:])
                q_sb = pool.tile([P, P], dtype=I8, tag="q")
                nc.vector.tensor_scalar_mul(q_sb[:], z_ps[:], qs_sb[:, 0:1])
                nc.sync.dma_start(out=hz[(G + g) * P:(G + g + 1) * P, :D],
                                  in_=q_sb[:])
                nc.sync.dma_start(out=hz[(G + g) * P:(G + g + 1) * P, D:],
                                  in_=s_sb[:].bitcast(I8))
    nc.compile()
    return nc


def _prepare_exec(nc):
    """AOT-compile the SPMD executable (mirrors run_bass_via_pjrt, but with
    lowering/compilation split out so the timed section is exec-only), and
    materialize the donated zero output buffers directly on device."""
    b2j.install_neuronx_cc_hook()
    partition_name = nc.partition_id_tensor.name if nc.partition_id_tensor else None
    in_names, out_names, out_avals, zero_shapes = [], [], [], []
    for alloc in nc.m.functions[0].allocations:
        if not isinstance(alloc, mybir.MemoryLocationSet):
            continue
        name = alloc.memorylocations[0].name
        if alloc.kind == "ExternalInput":
            if name != partition_name:
                in_names.append(name)
        elif alloc.kind == "ExternalOutput":
            out_names.append(name)
            shape = tuple(alloc.tensor_shape)
            dtype = mybir.dt.np(alloc.dtype)
            out_avals.append(jax.core.ShapedArray(shape, dtype))
            zero_shapes.append((shape, dtype))
    n_params = len(in_names)
    n_outs = len(out_avals)
    in_names = in_names + out_names
    if partition_name is not None:
        in_names.append(partition_name)
    donate = tuple(range(n_params, n_params + n_outs))

    def _body(*args):
        operands = list(args)
        if partition_name is not None:
            operands.append(b2j.partition_id_tensor())
        outs = b2j._bass_exec_p.bind(
            *operands, out_avals=tuple(out_avals), in_names=tuple(in_names),
            out_names=tuple(out_names), lowering_input_output_aliases=(),
            sim_require_finite=True, sim_require_nnan=True, nc=nc)
        return tuple(outs)

    devices = jax.devices()[:NCORES]
    mesh = Mesh(np.asarray(devices), ("core",))
    spec = PartitionSpec("core")
    in_specs = (spec,) * (n_params + n_outs)
    out_specs = (spec,) * n_outs
    sharded = jax.jit(
        shard_map(_body, mesh=mesh, in_specs=in_specs, out_specs=out_specs,
                  check_rep=False),
        donate_argnums=donate, keep_unused=True)

    def g_struct(shape, dtype):
        return jax.ShapeDtypeStruct((NCORES * shape[0], *shape[1:]), dtype)

    in_structs = []
    # parameter avals in declaration order, via the module allocations again
    shapes_by_name = {}
    for alloc in nc.m.functions[0].allocations:
        if isinstance(alloc, mybir.MemoryLocationSet) and alloc.kind == "ExternalInput":
            shapes_by_name[alloc.memorylocations[0].name] = (
                tuple(alloc.tensor_shape), mybir.dt.np(alloc.dtype))
    for name in in_names[:n_params]:
        shp, dt = shapes_by_name[name]
        in_structs.append(g_struct(shp, dt))
    zero_structs = [g_struct(shp, dt) for shp, dt in zero_shapes]
    compiled = sharded.lower(*in_structs, *zero_structs).compile()

    sharding = NamedSharding(mesh, spec)
    zeros_dev = [
        jax.jit(lambda s=shp, d=dt: jnp.zeros((NCORES * s[0], *s[1:]), d),
                out_shardings=sharding)()
        for shp, dt in zero_shapes]
    jax.block_until_ready(zeros_dev)
    return compiled, in_names[:n_params], out_names, out_avals, zeros_dev, sharding


def kernel(X, W0, W1, norm, src, dst):
    t0 = time.perf_counter()
    X = np.asarray(X, dtype=np.float32)
    W0 = np.asarray(W0, dtype=np.float32)
    W1 = np.asarray(W1, dtype=np.float32)
    norm = np.asarray(norm, dtype=np.float32)
    src = np.asarray(src).astype(np.int64)
    dst = np.asarray(dst).astype(np.int64)
    E = src.shape[0]

    # ---- host preprocessing: sort by dst, pack groups, shard to cores ----
    order = np.argsort(dst, kind="stable")
    src_s = src[order].astype(np.int32)
    dst_s = dst[order]
    norm_s = norm[order]
    groups = _pack_groups(dst_s)
    cum = np.cumsum([g[1] for g in groups])
    core_of = np.minimum((NCORES * (cum - 1) // E).astype(np.int64), NCORES - 1)
    per_core = [[] for _ in range(NCORES)]
    for gi, g in enumerate(groups):
        per_core[int(core_of[gi])].append(g)
    G = max(len(lst) for lst in per_core)
    G1 = G + 1

    # src remaps into the all-gathered padded layouts
    pos1 = ((src_s // RPC) * RPAD + (src_s % RPC)).astype(np.int32)
    pos2_map = np.full(N, G * P, dtype=np.int32)  # default: zero row

    pidx_arr = np.zeros((NCORES, G, P, TPG), dtype=np.int32)
    slot_arr = np.full((NCORES, G, P, TPG), -1, dtype=np.int8)
    sn_arr = np.zeros((NCORES, G, P, TPG), dtype=bfloat16)
    asm_rows, asm_ids = [], []
    for c in range(NCORES):
        rows_l, ids_l = [], []
        for g_i, (es, ce, node_ids) in enumerate(per_core[c]):
            d_loc = np.searchsorted(node_ids, dst_s[es:es + ce]).astype(np.float32)
            j = np.arange(ce)
            t_i, p_i = j // P, j % P
            pidx_arr[c, g_i, p_i, t_i] = pos1[es:es + ce]
            slot_arr[c, g_i, p_i, t_i] = d_loc.astype(np.int8)
            sn_arr[c, g_i, p_i, t_i] = norm_s[es:es + ce].astype(bfloat16)
            pos2_map[node_ids] = c * G1 * P + g_i * P + np.arange(len(node_ids))
            rows_l.append(g_i * P + np.arange(len(node_ids)))
            ids_l.append(node_ids)
        asm_rows.append(np.concatenate(rows_l) if rows_l else np.zeros(0, np.int64))
        asm_ids.append(np.concatenate(ids_l) if ids_l else np.zeros(0, np.int64))
    # layer-2 gathers use the same edge slots; pack both remaps in one int32
    pos2 = pos2_map[src_s]
    for c in range(NCORES):
        for g_i, (es, ce, node_ids) in enumerate(per_core[c]):
            j = np.arange(ce)
            pidx_arr[c, g_i, j % P, j // P] |= pos2[es:es + ce] << 16

    # quantize X per feature dim; fold the dequant scales into W0T rows so
    # the device never sees them
    xsc = (np.abs(X).max(axis=0) / 127.0 + 1e-20).astype(np.float32)
    Xq = np.round(X / xsc).astype(np.int8)
    W0T = np.ascontiguousarray(W0.T * xsc[:, None]).astype(bfloat16)
    W1T = np.ascontiguousarray(W1.T).astype(bfloat16)
    # per-core X shard, padded and transposed: [D, RPAD] int8
    Xpad = np.zeros((NCORES, RPAD, D), dtype=np.int8)
    Xpad[:, :RPC] = Xq.reshape(NCORES, RPC, D)
    XT = np.ascontiguousarray(Xpad.transpose(0, 2, 1))
    LAST_TIMES["prep_s"] = time.perf_counter() - t0

    t1 = time.perf_counter()
    nc = _build_fused(G)
    (compiled, in_names, out_names, out_avals, zeros_dev,
     sharding) = _prepare_exec(nc)
    LAST_TIMES["build_s"] = time.perf_counter() - t1

    per_core_in = {
        "xt": XT,
        "w0t": np.broadcast_to(W0T, (NCORES, D, D)),
        "w1t": np.broadcast_to(W1T, (NCORES, D, D)),
        "pidx": pidx_arr,
        "slot": slot_arr,
        "sn": sn_arr,
    }
    concat_in = [np.ascontiguousarray(per_core_in[name]).reshape(
        -1, *per_core_in[name].shape[2:]) for name in in_names]

    t1 = time.perf_counter()
    dev_in = jax.device_put(concat_in, [sharding] * len(concat_in))
    out_arrs = compiled(*dev_in, *zeros_dev)
    res = [np.asarray(a) for a in out_arrs]
    LAST_TIMES["run_fused_s"] = time.perf_counter() - t1

    hz_q = res[out_names.index("hz")].reshape(NCORES, 2 * G * P, D + 4)
    hsc = np.ascontiguousarray(hz_q[:, :, D:]).view(np.float32)
    H = np.zeros((N, D), dtype=np.float32)
    Z = np.zeros((N, D), dtype=np.float32)
    for c in range(NCORES):
        hz_f = hz_q[c, :, :D].astype(np.float32) * hsc[c]
        H[asm_ids[c]] = hz_f[:G * P][asm_rows[c]]
        zc = hz_f[G * P:].reshape(G, P, P).transpose(0, 2, 1).reshape(G * P, P)
        Z[asm_ids[c]] = zc[asm_rows[c]]

    LAST_TIMES["total_s"] = time.perf_counter() - t0
    return (Z, H)


# revision 21
# speedup vs baseline: 1.4568x; 1.0254x over previous
"""2-layer GCN on 8 trn2 NeuronCores — single fused SPMD launch.

Full inputs in, full outputs out. Host sorts edges by dst and packs them
into groups of <=128 dst-nodes / <=2048 edges (16 tiles of 128). Each core
owns a contiguous run of groups (balanced by edge count) plus 1/8 of the
nodes for the dense layer. Per-tile segment-sum is a TensorE matmul with an
on-device-built one-hot*(norm) selection matrix, accumulated in PSUM.

One launch does everything on device:
  A: S0_c = X_c @ W0.T          (node-sharded)      -> AllGather S0
  B: H_c  = relu(seg_sum(S0[src]*norm, dst))        -> AllGather H
  C: Z_c  = seg_sum(H[src]*norm, dst) @ W1.T        (stored transposed)

src indices are pre-remapped on the host into positions in the
all-gathered (padded, core-major) S0/H layouts; the two remaps are packed
into one int32 (layer1 | layer2<<16) and unpacked on device.

Host<->device tunnel traffic dominates wall time (~40-50 MB/s, ~0.08 s
fixed cost per array), so every stream is squeezed: X ships int8 with
per-feature-dim scales folded into W0.T, slot ids ship int8, norms bf16,
on-device intermediates are bf16, and the fused H/Z output is int8 with
per-row f32 scales bit-packed into 4 trailing byte columns of the single
output tensor. The PJRT executable is AOT-compiled at build time
(persistent jax compilation cache + neuron NEFF cache make this fast on
repeat runs); the timed section is transfer + execute + readback only.
"""

import os
import time

import numpy as np
from ml_dtypes import bfloat16

import jax

jax.config.update("jax_compilation_cache_dir",
                  os.path.expanduser("~/.jax_comp_cache"))
jax.config.update("jax_persistent_cache_min_entry_size_bytes", -1)
jax.config.update("jax_persistent_cache_min_compile_time_secs", 0)

import jax.numpy as jnp
from jax.sharding import Mesh, NamedSharding, PartitionSpec
from jax.experimental.shard_map import shard_map

import concourse.bacc as bacc
import concourse.bass as bass
import concourse.bass2jax as b2j
import concourse.tile as tile
from concourse import mybir

P = 128
TPG = 16                 # tiles (of 128 edges) per group
EPG = P * TPG            # 2048 edge slots per group
NCORES = 8
N = 50000
D = 128
RPC = N // NCORES        # 6250 node rows per core (exact)
CHA = -(-RPC // P)       # 49 row-tiles per core in phase A
RPAD = CHA * P           # 6272 padded rows per core
F32 = mybir.dt.float32
BF16 = mybir.dt.bfloat16
I32 = mybir.dt.int32
I8 = mybir.dt.int8

LAST_TIMES = {}


def _pack_groups(dst_sorted):
    """Greedy pack sorted dst nodes into groups (<=P nodes, <=EPG edges).
    Returns list of (edge_start, edge_cnt, node_ids ndarray)."""
    nodes, counts = np.unique(dst_sorted, return_counts=True)
    groups = []
    i, e = 0, 0
    nn = len(nodes)
    while i < nn:
        es = e
        ns = i
        cnt_e = 0
        while i < nn and (i - ns) < P and cnt_e + counts[i] <= EPG:
            cnt_e += int(counts[i])
            i += 1
        assert i > ns, "single node exceeds group capacity"
        e += cnt_e
        groups.append((es, cnt_e, nodes[ns:i]))
    return groups


def _build_fused(G):
    """G = max real groups per core. h_loc gets one extra all-zero group so
    its first row doubles as the gather target for srcs with no in-edges."""
    G1 = G + 1
    # disable_frame_to_traceback keeps python source locations out of the
    # emitted program, so the NEFF/XLA caches stay warm across file edits
    # and directory moves
    nc = bacc.Bacc(None, target_bir_lowering=False, num_swdge_queues=4,
                   num_devices=NCORES, disable_frame_to_traceback=True)
    # X ships int8, quantized per feature dim; the dequant scales are folded
    # into w0t rows on the host so the device only does an i8->bf16 copy
    xt = nc.declare_dram_parameter("xt", [D, RPAD], I8, isOutput=False)
    w0t = nc.declare_dram_parameter("w0t", [D, D], BF16, isOutput=False)
    w1t = nc.declare_dram_parameter("w1t", [D, D], BF16, isOutput=False)
    pidx = nc.declare_dram_parameter("pidx", [G, P, TPG], I32, isOutput=False)
    slot = nc.declare_dram_parameter("slot", [G, P, TPG], I8, isOutput=False)
    sn = nc.declare_dram_parameter("sn", [G, P, TPG], BF16, isOutput=False)
    # single fused output (H rows then Z.T rows), int8 with per-row f32
    # scales bit-packed into 4 trailing byte columns: halves the dominant
    # d2h stream vs bf16 at ~0.7% added error, in one contiguous transfer
    hz = nc.declare_dram_parameter("hz", [2 * G * P, D + 4], I8, isOutput=True)

    with tile.TileContext(nc) as tc:
        with (
            tc.tile_pool(name="dram", bufs=1, space="DRAM") as dram,
            tc.tile_pool(name="const", bufs=1) as cpool,
            tc.tile_pool(name="sbuf", bufs=4) as pool,
            tc.tile_pool(name="psum", bufs=2, space="PSUM") as psum,
            tc.tile_pool(name="psum2", bufs=2, space="PSUM") as psum2,
        ):
            s0_loc = dram.tile([RPAD, D], BF16)
            s0_full = dram.tile([NCORES * RPAD, D], BF16)
            h_loc = dram.tile([G1 * P, D], BF16)
            h_full = dram.tile([NCORES * G1 * P, D], BF16)

            iota_i = cpool.tile([P, P], dtype=I32)
            nc.gpsimd.iota(iota_i[:], pattern=[[1, P]], base=0,
                           channel_multiplier=0)
            iota_sb = cpool.tile([P, P], dtype=BF16)
            nc.vector.tensor_copy(iota_sb[:], iota_i[:])
            zrow_sb = cpool.tile([P, D], dtype=BF16)
            nc.vector.memset(zrow_sb[:], 0.0)
            w0t_sb = cpool.tile([D, D], dtype=BF16)
            nc.sync.dma_start(out=w0t_sb[:], in_=w0t[:])
            w1t_sb = cpool.tile([D, D], dtype=BF16)
            nc.sync.dma_start(out=w1t_sb[:], in_=w1t[:])

            # ---- phase A: S0_c = X_c @ W0.T (X arrives transposed) ----
            for t in range(CHA):
                xq_sb = pool.tile([P, P], dtype=I8, tag="xq")
                nc.sync.dma_start(out=xq_sb[:], in_=xt[:, t * P:(t + 1) * P])
                xt_sb = pool.tile([P, P], dtype=BF16, tag="xt")
                nc.vector.tensor_copy(xt_sb[:], xq_sb[:])
                s_ps = psum.tile([P, D], dtype=F32, tag="s")
                nc.tensor.matmul(out=s_ps[:], lhsT=xt_sb[:], rhs=w0t_sb[:],
                                 start=True, stop=True)
                s_sb = pool.tile([P, D], dtype=BF16, tag="s0")
                nc.vector.tensor_copy(s_sb[:], s_ps[:])
                nc.sync.dma_start(out=s0_loc[t * P:(t + 1) * P, :], in_=s_sb[:])

            nc.gpsimd.collective_compute(
                "AllGather", mybir.AluOpType.bypass,
                replica_groups=[list(range(NCORES))],
                ins=[s0_loc[:].opt()], outs=[s0_full[:].opt()],
            )

            # ---- phase B: H = relu(seg_sum(S0[src]*norm, dst)) ----
            nc.sync.dma_start(out=h_loc[G * P:G1 * P, :], in_=zrow_sb[:])
            for g in range(G):
                pidx_sb = pool.tile([P, TPG], dtype=I32, tag="pidx")
                nc.sync.dma_start(out=pidx_sb[:], in_=pidx[g])
                idx_sb = pool.tile([P, TPG], dtype=I32, tag="idx")
                nc.vector.tensor_scalar(
                    out=idx_sb[:], in0=pidx_sb[:], scalar1=0xFFFF, scalar2=None,
                    op0=mybir.AluOpType.bitwise_and)
                sl8_sb = pool.tile([P, TPG], dtype=I8, tag="sl8")
                nc.sync.dma_start(out=sl8_sb[:], in_=slot[g])
                sl_sb = pool.tile([P, TPG], dtype=BF16, tag="sl")
                nc.vector.tensor_copy(sl_sb[:], sl8_sb[:])
                sn_sb = pool.tile([P, TPG], dtype=BF16, tag="sn")
                nc.sync.dma_start(out=sn_sb[:], in_=sn[g])
                nrm_sb = pool.tile([P, TPG], dtype=F32, tag="nrm")
                nc.vector.tensor_copy(nrm_sb[:], sn_sb[:])
                acc_ps = psum.tile([P, D], dtype=F32, tag="acc")
                for t in range(TPG):
                    g_sb = pool.tile([P, D], dtype=BF16, tag="gat")
                    nc.gpsimd.indirect_dma_start(
                        out=g_sb[:], out_offset=None, in_=s0_full[:],
                        in_offset=bass.IndirectOffsetOnAxis(
                            ap=idx_sb[:, t:t + 1], axis=0),
                    )
                    sel = pool.tile([P, P], dtype=BF16, tag="sel")
                    nc.vector.tensor_tensor(
                        out=sel[:], in0=sl_sb[:, t:t + 1].to_broadcast([P, P])[:],
                        in1=iota_sb[:], op=mybir.AluOpType.is_equal,
                    )
                    pm = pool.tile([P, P], dtype=BF16, tag="pm")
                    nc.vector.tensor_scalar_mul(
                        pm[:], sel[:], nrm_sb[:, t:t + 1])
                    nc.tensor.matmul(out=acc_ps[:], lhsT=pm[:], rhs=g_sb[:],
                                     start=(t == 0), stop=(t == TPG - 1))
                h_sb = pool.tile([P, D], dtype=BF16, tag="h")
                nc.scalar.activation(h_sb[:], acc_ps[:],
                                     mybir.ActivationFunctionType.Relu)
                nc.sync.dma_start(out=h_loc[g * P:(g + 1) * P, :], in_=h_sb[:])
                # int8-quantize H rows (relu output >= 0, so max == absmax)
                m_sb = pool.tile([P, 1], dtype=F32, tag="m")
                nc.vector.reduce_max(m_sb[:], h_sb[:], axis=mybir.AxisListType.X)
                s_sb = pool.tile([P, 1], dtype=F32, tag="s")
                nc.scalar.activation(s_sb[:], m_sb[:],
                                     mybir.ActivationFunctionType.Copy,
                                     bias=1e-20, scale=1.0 / 127.0)
                qs_sb = pool.tile([P, 1], dtype=F32, tag="qs")
                nc.vector.reciprocal(qs_sb[:], s_sb[:])
                q_sb = pool.tile([P, D], dtype=I8, tag="q")
                nc.vector.tensor_scalar_mul(q_sb[:], h_sb[:], qs_sb[:, 0:1])
                nc.sync.dma_start(out=hz[g * P:(g + 1) * P, :D], in_=q_sb[:])
                nc.sync.dma_start(out=hz[g * P:(g + 1) * P, D:],
                                  in_=s_sb[:].bitcast(I8))

            nc.gpsimd.collective_compute(
                "AllGather", mybir.AluOpType.bypass,
                replica_groups=[list(range(NCORES))],
                ins=[h_loc[:].opt()], outs=[h_full[:].opt()],
            )

            # ---- phase C: Z = seg_sum(H[src]*norm, dst) @ W1.T ----
            # Accumulate transposed (accT = gathered.T @ pm) so the final
            # matmul zT = w1t.T @ accT needs no PE transpose. zout holds
            # Z_g.T per group; the host transposes back.
            for g in range(G):
                pidx_sb = pool.tile([P, TPG], dtype=I32, tag="pidx")
                nc.sync.dma_start(out=pidx_sb[:], in_=pidx[g])
                idx_sb = pool.tile([P, TPG], dtype=I32, tag="idx")
                nc.vector.tensor_scalar(
                    out=idx_sb[:], in0=pidx_sb[:], scalar1=16, scalar2=None,
                    op0=mybir.AluOpType.logical_shift_right)
                sl8_sb = pool.tile([P, TPG], dtype=I8, tag="sl8")
                nc.sync.dma_start(out=sl8_sb[:], in_=slot[g])
                sl_sb = pool.tile([P, TPG], dtype=BF16, tag="sl")
                nc.vector.tensor_copy(sl_sb[:], sl8_sb[:])
                sn_sb = pool.tile([P, TPG], dtype=BF16, tag="sn")
                nc.sync.dma_start(out=sn_sb[:], in_=sn[g])
                nrm_sb = pool.tile([P, TPG], dtype=F32, tag="nrm")
                nc.vector.tensor_copy(nrm_sb[:], sn_sb[:])
                acc_ps = psum.tile([P, P], dtype=F32, tag="acc")
                for t in range(TPG):
                    g_sb = pool.tile([P, D], dtype=BF16, tag="gat")
                    nc.gpsimd.indirect_dma_start(
                        out=g_sb[:], out_offset=None, in_=h_full[:],
                        in_offset=bass.IndirectOffsetOnAxis(
                            ap=idx_sb[:, t:t + 1], axis=0),
                    )
                    sel = pool.tile([P, P], dtype=BF16, tag="sel")
                    nc.vector.tensor_tensor(
                        out=sel[:], in0=sl_sb[:, t:t + 1].to_broadcast([P, P])[:],
                        in1=iota_sb[:], op=mybir.AluOpType.is_equal,
                    )
                    pm = pool.tile([P, P], dtype=BF16, tag="pm")
                    nc.vector.tensor_scalar_mul(
                        pm[:], sel[:], nrm_sb[:, t:t + 1])
                    nc.tensor.matmul(out=acc_ps[:], lhsT=g_sb[:], rhs=pm[:],
                                     start=(t == 0), stop=(t == TPG - 1))
                at_sb = pool.tile([P, P], dtype=BF16, tag="aT")
                nc.vector.tensor_copy(at_sb[:], acc_ps[:])
                z_ps = psum2.tile([P, P], dtype=F32, tag="zT")
                nc.tensor.matmul(out=z_ps[:], lhsT=w1t_sb[:], rhs=at_sb[:],
                                 start=True, stop=True)
                # int8-quantize Z.T rows (per out-dim within the group)
                m_sb = pool.tile([P, 1], dtype=F32, tag="m")
                nc.vector.reduce_max(m_sb[:], z_ps[:], axis=mybir.AxisListType.X,
                                     apply_absolute_value=True)
                s_sb = pool.tile([P, 1], dtype=F32, tag="s")
                nc.scalar.activation(s_sb[:], m_sb[:],
                                     mybir.ActivationFunctionType.Copy,
                                     bias=1e-20, scale=1.0 / 127.0)
                qs_sb = pool.tile([P, 1], dtype=F32, tag="qs")
                nc.vector.reciprocal(qs_sb[:], s_sb[:])
                q_sb = pool.tile([P, P], dtype=I8, tag="q")
                nc.vector.tensor_scalar_mul(q_sb[:], z_ps[:], qs_sb[:, 0:1])
                nc.sync.dma_start(out=hz[(G + g) * P:(G + g + 1) * P, :D],
                                  in_=q_sb[:])
                nc.sync.dma_start(out=hz[(G + g) * P:(G + g + 1) * P, D:],
                                  in_=s_sb[:].bitcast(I8))
    nc.compile()
    return nc


def _prepare_exec(nc):
    """AOT-compile the SPMD executable (mirrors run_bass_via_pjrt, but with
    lowering/compilation split out so the timed section is exec-only), and
    materialize the donated zero output buffers directly on device."""
    b2j.install_neuronx_cc_hook()
    partition_name = nc.partition_id_tensor.name if nc.partition_id_tensor else None
    in_names, out_names, out_avals, zero_shapes = [], [], [], []
    for alloc in nc.m.functions[0].allocations:
        if not isinstance(alloc, mybir.MemoryLocationSet):
            continue
        name = alloc.memorylocations[0].name
        if alloc.kind == "ExternalInput":
            if name != partition_name:
                in_names.append(name)
        elif alloc.kind == "ExternalOutput":
            out_names.append(name)
            shape = tuple(alloc.tensor_shape)
            dtype = mybir.dt.np(alloc.dtype)
            out_avals.append(jax.core.ShapedArray(shape, dtype))
            zero_shapes.append((shape, dtype))
    n_params = len(in_names)
    n_outs = len(out_avals)
    in_names = in_names + out_names
    if partition_name is not None:
        in_names.append(partition_name)
    donate = tuple(range(n_params, n_params + n_outs))

    def _body(*args):
        operands = list(args)
        if partition_name is not None:
            operands.append(b2j.partition_id_tensor())
        outs = b2j._bass_exec_p.bind(
            *operands, out_avals=tuple(out_avals), in_names=tuple(in_names),
            out_names=tuple(out_names), lowering_input_output_aliases=(),
            sim_require_finite=True, sim_require_nnan=True, nc=nc)
        return tuple(outs)

    devices = jax.devices()[:NCORES]
    mesh = Mesh(np.asarray(devices), ("core",))
    spec = PartitionSpec("core")
    in_specs = (spec,) * (n_params + n_outs)
    out_specs = (spec,) * n_outs
    sharded = jax.jit(
        shard_map(_body, mesh=mesh, in_specs=in_specs, out_specs=out_specs,
                  check_rep=False),
        donate_argnums=donate, keep_unused=True)

    def g_struct(shape, dtype):
        return jax.ShapeDtypeStruct((NCORES * shape[0], *shape[1:]), dtype)

    in_structs = []
    # parameter avals in declaration order, via the module allocations again
    shapes_by_name = {}
    for alloc in nc.m.functions[0].allocations:
        if isinstance(alloc, mybir.MemoryLocationSet) and alloc.kind == "ExternalInput":
            shapes_by_name[alloc.memorylocations[0].name] = (
                tuple(alloc.tensor_shape), mybir.dt.np(alloc.dtype))
    for name in in_names[:n_params]:
        shp, dt = shapes_by_name[name]
        in_structs.append(g_struct(shp, dt))
    zero_structs = [g_struct(shp, dt) for shp, dt in zero_shapes]
    compiled = sharded.lower(*in_structs, *zero_structs).compile()

    sharding = NamedSharding(mesh, spec)
    zeros_dev = [
        jax.jit(lambda s=shp, d=dt: jnp.zeros((NCORES * s[0], *s[1:]), d),
                out_shardings=sharding)()
        for shp, dt in zero_shapes]
    jax.block_until_ready(zeros_dev)
    return compiled, in_names[:n_params], out_names, out_avals, zeros_dev, sharding


def kernel(X, W0, W1, norm, src, dst):
    t0 = time.perf_counter()
    X = np.asarray(X, dtype=np.float32)
    W0 = np.asarray(W0, dtype=np.float32)
    W1 = np.asarray(W1, dtype=np.float32)
    norm = np.asarray(norm, dtype=np.float32)
    src = np.asarray(src).astype(np.int64)
    dst = np.asarray(dst).astype(np.int64)
    E = src.shape[0]

    # ---- host preprocessing: sort by dst, pack groups, shard to cores ----
    order = np.argsort(dst, kind="stable")
    src_s = src[order].astype(np.int32)
    dst_s = dst[order]
    norm_s = norm[order]
    groups = _pack_groups(dst_s)
    cum = np.cumsum([g[1] for g in groups])
    core_of = np.minimum((NCORES * (cum - 1) // E).astype(np.int64), NCORES - 1)
    per_core = [[] for _ in range(NCORES)]
    for gi, g in enumerate(groups):
        per_core[int(core_of[gi])].append(g)
    G = max(len(lst) for lst in per_core)
    G1 = G + 1

    # src remaps into the all-gathered padded layouts
    pos1 = ((src_s // RPC) * RPAD + (src_s % RPC)).astype(np.int32)
    pos2_map = np.full(N, G * P, dtype=np.int32)  # default: zero row

    pidx_arr = np.zeros((NCORES, G, P, TPG), dtype=np.int32)
    slot_arr = np.full((NCORES, G, P, TPG), -1, dtype=np.int8)
    sn_arr = np.zeros((NCORES, G, P, TPG), dtype=bfloat16)
    asm_rows, asm_ids = [], []
    for c in range(NCORES):
        rows_l, ids_l = [], []
        for g_i, (es, ce, node_ids) in enumerate(per_core[c]):
            d_loc = np.searchsorted(node_ids, dst_s[es:es + ce]).astype(np.float32)
            j = np.arange(ce)
            t_i, p_i = j // P, j % P
            pidx_arr[c, g_i, p_i, t_i] = pos1[es:es + ce]
            slot_arr[c, g_i, p_i, t_i] = d_loc.astype(np.int8)
            sn_arr[c, g_i, p_i, t_i] = norm_s[es:es + ce].astype(bfloat16)
            pos2_map[node_ids] = c * G1 * P + g_i * P + np.arange(len(node_ids))
            rows_l.append(g_i * P + np.arange(len(node_ids)))
            ids_l.append(node_ids)
        asm_rows.append(np.concatenate(rows_l) if rows_l else np.zeros(0, np.int64))
        asm_ids.append(np.concatenate(ids_l) if ids_l else np.zeros(0, np.int64))
    # layer-2 gathers use the same edge slots; pack both remaps in one int32
    pos2 = pos2_map[src_s]
    for c in range(NCORES):
        for g_i, (es, ce, node_ids) in enumerate(per_core[c]):
            j = np.arange(ce)
            pidx_arr[c, g_i, j % P, j // P] |= pos2[es:es + ce] << 16

    # quantize X per feature dim; fold the dequant scales into W0T rows so
    # the device never sees them
    xsc = (np.abs(X).max(axis=0) / 127.0 + 1e-20).astype(np.float32)
    Xq = np.round(X / xsc).astype(np.int8)
    W0T = np.ascontiguousarray(W0.T * xsc[:, None]).astype(bfloat16)
    W1T = np.ascontiguousarray(W1.T).astype(bfloat16)
    # per-core X shard, padded and transposed: [D, RPAD] int8
    Xpad = np.zeros((NCORES, RPAD, D), dtype=np.int8)
    Xpad[:, :RPC] = Xq.reshape(NCORES, RPC, D)
    XT = np.ascontiguousarray(Xpad.transpose(0, 2, 1))
    LAST_TIMES["prep_s"] = time.perf_counter() - t0

    t1 = time.perf_counter()
    nc = _build_fused(G)
    (compiled, in_names, out_names, out_avals, zeros_dev,
     sharding) = _prepare_exec(nc)
    LAST_TIMES["build_s"] = time.perf_counter() - t1

    per_core_in = {
        "xt": XT,
        "w0t": np.broadcast_to(W0T, (NCORES, D, D)),
        "w1t": np.broadcast_to(W1T, (NCORES, D, D)),
        "pidx": pidx_arr,
        "slot": slot_arr,
        "sn": sn_arr,
    }
    concat_in = [np.ascontiguousarray(per_core_in[name]).reshape(
        -1, *per_core_in[name].shape[2:]) for name in in_names]

    t1 = time.perf_counter()
    dev_in = jax.device_put(concat_in, [sharding] * len(concat_in))
    out_arrs = compiled(*dev_in, *zeros_dev)
    res = [np.asarray(a) for a in out_arrs]
    LAST_TIMES["run_fused_s"] = time.perf_counter() - t1

    hz_q = res[out_names.index("hz")].reshape(NCORES, 2 * G * P, D + 4)
    hsc = np.ascontiguousarray(hz_q[:, :, D:]).view(np.float32)
    H = np.zeros((N, D), dtype=np.float32)
    Z = np.zeros((N, D), dtype=np.float32)
    for c in range(NCORES):
        hz_f = hz_q[c, :, :D].astype(np.float32) * hsc[c]
        H[asm_ids[c]] = hz_f[:G * P][asm_rows[c]]
        zc = hz_f[G * P:].reshape(G, P, P).transpose(0, 2, 1).reshape(G * P, P)
        Z[asm_ids[c]] = zc[asm_rows[c]]

    LAST_TIMES["total_s"] = time.perf_counter() - t0
    return (Z, H)
